# revision 2
# baseline (speedup 1.0000x reference)
"""Multi-head causal self-attention (B=2, S=2048, D=1024, H=16) on 8 TRN2
NeuronCores via Bass/Tile.

Sharding: core c -> (batch b = c // 4, head-group g = c % 4). Each core
computes q/k/v projections for its 4 heads (256 of 1024 projection cols),
causal flash attention for those heads, and a partial output projection
(row-parallel over the head dim). Host sums the 4 partials per batch.

Device layouts (all transposed so the contraction dim sits on partitions):
  xT   [D, S]   : x[b].T, host-transposed
  Q^T/K^T [e, S]: head dim on partitions
  V    [k, e+1] : natural, with a ones column per head; the ones column turns
                  the AV^T matmul into (unnormalized AV^T, softmax denom) rows
  A^T  [e, S]   : produced directly by AV^T matmul, consumed as moving
                  operand of the output projection -> zero on-chip transposes
  outT [D, S]   : transposed partial output, host sums + transposes back

Engine split: PE all matmuls (incl. the reciprocal partition-broadcast as a
K=1 outer product), ACT *only* Exp (avoids activation-table switches), DVE
all psum evictions / masking / normalization, chunked HWDGE DMAs so compute
starts as soon as the first d-chunk lands.

Scores are computed as S^T[k, q] = (K^T_blk)^T @ Q^T so softmax reduces over
the partition dim (folded into the AV matmul via the ones column). exp()
needs no max-subtraction: scores are O(1) here, f32 exp is exact enough.
Matmul operands use float32r (replicated-weight fp32: full PE rate at
moving>=256, ~19-bit mantissa) -> rel err ~3e-4 vs fp32 reference.
"""

from contextlib import ExitStack

import numpy as np

import concourse.bass as bass
import concourse.mybir as mybir
import concourse.tile as tile
from concourse.bass_utils import run_bass_kernel_spmd

# Problem constants (hardcoded per harness contract).
B, S, D, NH, DH = 2, 2048, 1024, 16, 64
N_CORES = 8
GROUPS = 4                 # head-groups; cores per batch
HPC = NH // GROUPS         # heads per core = 4
E = HPC * DH               # per-core projection width = 256
P = 128                    # SBUF partitions
SC = 512                   # moving-operand chunk (q chunk)
ND = D // P                # 8 d-chunks
NEB = E // P               # 2 e-blocks per core
NQ = S // SC               # 4 q chunks
NKB = S // P               # 16 k blocks
SCALE = DH ** -0.5

F32 = mybir.dt.float32
MM_DT = mybir.dt.float32r


def _split_multiwait(nc, max_waits=1):
    """This toolchain's walrus codegen accepts at most one sync-wait per
    instruction ("Too many sync wait commands"). Tile emits multi-wait
    instructions (notably the kernel-tail Drain). Keep the last wait (+ all
    updates) on the original instruction and hoist earlier waits onto
    single-wait Drains inserted before it on the same engine."""
    for f in nc.m.functions:
        for bb in f.blocks:
            new = []
            changed = False
            for inst in bb.instructions:
                si = inst.sync_info
                waits = list(si.on_wait) if si is not None and si.on_wait else []
                if len(waits) > max_waits:
                    for j, w in enumerate(waits[:-max_waits]):
                        d = mybir.InstDrain(name=f"{inst.name}-sw{j}", ins=[], outs=[])
                        d.engine = inst.engine
                        d.sync_info = mybir.SyncInfo(on_wait=[w], on_update=[])
                        new.append(d)
                    inst.sync_info = mybir.SyncInfo(
                        on_wait=waits[-max_waits:],
                        on_update=list(si.on_update) if si.on_update else [],
                    )
                    changed = True
                new.append(inst)
            if changed:
                bb.instructions = new


def build_nc(repeat=1, pav_bufs=4, psc_bufs=3, pmx_bufs=1, ptp_bufs=8):
    """repeat>1 wraps the whole body in a hardware For_i loop — used only by
    the benchmark to amortize dispatch overhead out of wall-clock timing."""
    nc = bass.Bass("TRN2", target_bir_lowering=False, debug=False,
                   num_devices=N_CORES)

    xT = nc.dram_tensor("xT", [D, S], MM_DT, kind="ExternalInput")
    wqT = nc.dram_tensor("wqT", [D, E], MM_DT, kind="ExternalInput")
    wkT = nc.dram_tensor("wkT", [D, E], MM_DT, kind="ExternalInput")
    wvT = nc.dram_tensor("wvT", [D, E], MM_DT, kind="ExternalInput")
    woT = nc.dram_tensor("woT", [E, D], MM_DT, kind="ExternalInput")
    bq = nc.dram_tensor("bq", [E], F32, kind="ExternalInput")
    bk = nc.dram_tensor("bk", [E], F32, kind="ExternalInput")
    outT = nc.dram_tensor("outT", [D, S], F32, kind="ExternalOutput")

    AF = mybir.ActivationFunctionType
    with tile.TileContext(nc) as tc:
        with ExitStack() as ctx:
            if repeat > 1:
                ctx.enter_context(tc.For_i(0, repeat, 1))
            const = ctx.enter_context(tc.tile_pool(name="const", bufs=1))

            # ---- persistent SBUF tensors (chunked for fine-grained deps) ----
            x_sbs = [const.tile([P, S], MM_DT, tag=f"x{i}", name=f"x{i}") for i in range(ND)]
            wq_sbs = [const.tile([P, E], MM_DT, tag=f"wq{i}", name=f"wq{i}") for i in range(ND)]
            wk_sbs = [const.tile([P, E], MM_DT, tag=f"wk{i}", name=f"wk{i}") for i in range(ND)]
            wv_sbs = [const.tile([P, E], MM_DT, tag=f"wv{i}", name=f"wv{i}") for i in range(ND)]
            wo_sbs = [const.tile([P, D], MM_DT, tag=f"wo{i}", name=f"wo{i}") for i in range(NEB)]
            bq_sb = const.tile([P, NEB], F32, tag="bq", name="bq")
            bk_sb = const.tile([P, NEB], F32, tag="bk", name="bk")
            # Q^T/K^T per (e-block, q-chunk); V per 512-wide k-chunk
            qts = [[const.tile([P, SC], MM_DT, tag=f"qt{e}{c}", name=f"qt{e}{c}") for c in range(NQ)]
                   for e in range(NEB)]
            kts = [[const.tile([P, SC], MM_DT, tag=f"kt{e}{c}", name=f"kt{e}{c}") for c in range(NQ)]
                   for e in range(NEB)]
            v_sbs = [const.tile([P, NQ, HPC * (DH + 1)], MM_DT, tag=f"v{i}", name=f"v{i}")
                     for i in range(NQ)]
            at_sbs = [[const.tile([P, SC], MM_DT, tag=f"at{i}{f}", name=f"at{i}{f}")
                       for f in range(NEB)] for i in range(NQ)]
            mk_sb = const.tile([P, NQ, SC], MM_DT, tag="mk", name="mk")
            ones_sb = const.tile([1, DH], MM_DT, tag="ones", name="ones")

            # input DMAs interleaved per d-chunk so the first projection
            # matmuls start as soon as (w0, x0) land
            for di in range(ND):
                nc.sync.dma_start(wk_sbs[di][:], wkT[di * P:(di + 1) * P, :])
                nc.sync.dma_start(wq_sbs[di][:], wqT[di * P:(di + 1) * P, :])
                nc.sync.dma_start(wv_sbs[di][:], wvT[di * P:(di + 1) * P, :])
                # alternate x chunks across HWDGE/SWDGE queues for
                # parallelism (SWDGE inside For_i fails codegen, so the
                # benchmark repeat-loop build uses HWDGE only)
                use_sw = (di % 2 == 1) and repeat == 1
                dma = nc.gpsimd.dma_start if use_sw else nc.sync.dma_start
                dma(x_sbs[di][:], xT[di * P:(di + 1) * P, :])
            for ft in range(NEB):
                nc.sync.dma_start(wo_sbs[ft][:], woT[ft * P:(ft + 1) * P, :])
            nc.sync.dma_start(bq_sb[:], bq.rearrange("(n p) -> p n", p=P))
            nc.sync.dma_start(bk_sb[:], bk.rearrange("(n p) -> p n", p=P))

            # constants: ones + multiplicative causal masks
            # (affine_select/memset can't write f32r; build f32, DVE-round)
            tmp = ctx.enter_context(tc.tile_pool(name="tmp", bufs=1))
            one_f32 = tmp.tile([P, 1], F32, tag="onef", name="onef")
            nc.vector.memset(one_f32[:], 1.0)
            nc.vector.tensor_copy(ones_sb[:],
                                  one_f32[0:1, 0:1].broadcast_to([1, DH]))
            # mk[m][kk, qq] = 1.0 if kk + 128*m <= qq else 0.0
            mkf_sb = tmp.tile([P, NQ, SC], F32, tag="mkf", name="mkf")
            for m in range(NQ):
                nc.gpsimd.memset(mkf_sb[:, m, :], 1.0)
                nc.gpsimd.affine_select(
                    out=mkf_sb[:, m, :], in_=mkf_sb[:, m, :],
                    compare_op=mybir.AluOpType.is_ge, fill=0.0,
                    base=-(P * m), pattern=[[1, SC]], channel_multiplier=-1,
                )
            nc.vector.tensor_copy(mk_sb[:], mkf_sb[:])
            for cc in range(NQ):
                nc.vector.tensor_copy(
                    v_sbs[cc][:, :, DH::DH + 1],
                    one_f32[:, :, None].broadcast_to([P, NQ, HPC]))

            # Dedicated PSUM pools so long-lived AV accumulators can't
            # starve score/projection/output tiles (8 banks total).
            pav = ctx.enter_context(tc.tile_pool(name="pav", bufs=pav_bufs, space="PSUM"))
            psc = ctx.enter_context(tc.tile_pool(name="psc", bufs=psc_bufs, space="PSUM"))
            pmx = ctx.enter_context(tc.tile_pool(name="pmx", bufs=pmx_bufs, space="PSUM"))
            ptp = ctx.enter_context(tc.tile_pool(name="ptp", bufs=ptp_bufs))
            rcp = ctx.enter_context(tc.tile_pool(name="rcp", bufs=2))
            obp = ctx.enter_context(tc.tile_pool(name="obp", bufs=3))

            def project_qk(w_sbs, b_sb, o_tiles, c):
                for eb in range(NEB):
                    ps = psc.tile([P, SC], F32, tag="sc", name="sc")
                    for di in range(ND):
                        nc.tensor.matmul(
                            ps[:],
                            lhsT=w_sbs[di][:, eb * P:(eb + 1) * P],
                            rhs=x_sbs[di][:, c * SC:(c + 1) * SC],
                            start=(di == 0), stop=(di == ND - 1),
                        )
                    nc.vector.tensor_scalar_add(
                        out=o_tiles[eb][c][:], in0=ps[:],
                        scalar1=b_sb[:, eb:eb + 1])

            def project_v(c):
                for kk in range(NQ):          # 4 k-blocks per 512 chunk
                    kb = c * NQ + kk
                    ps = psc.tile([P, SC], F32, tag="sc", name="sc")
                    for di in range(ND):
                        nc.tensor.matmul(
                            ps[:, :E],
                            lhsT=x_sbs[di][:, kb * P:(kb + 1) * P],
                            rhs=wv_sbs[di][:],
                            start=(di == 0), stop=(di == ND - 1),
                        )
                    dst = v_sbs[c][:, kk, :].rearrange(
                        "p (h e) -> p h e", h=HPC)[:, :, :DH]
                    nc.vector.tensor_copy(
                        dst, ps[:, :E].rearrange("p (h e) -> p h e", h=HPC))

            def attention(h, c):
                et, er = h // 2, (h % 2) * DH
                av = pav.tile([DH + 1, SC], F32, tag="av", name="av")
                nkb_c = NQ * (c + 1)
                for j in range(nkb_c):
                    m = j - NQ * c
                    # diagonal block at offset m: columns qq < 128m are fully
                    # masked -- restrict all work to [:, q0:]
                    q0 = P * m if m > 0 else 0
                    ps = psc.tile([P, SC], F32, tag="sc", name="sc")
                    nc.tensor.matmul(
                        ps[:, q0:],
                        lhsT=kts[et][j // NQ][er:er + DH,
                                              (j % NQ) * P:(j % NQ + 1) * P],
                        rhs=qts[et][c][er:er + DH, q0:],
                        start=True, stop=True,
                    )
                    pt = ptp.tile([P, SC], MM_DT, tag="pt", name="pt")
                    nc.scalar.activation(pt[:, q0:], ps[:, q0:], AF.Exp,
                                         scale=SCALE)
                    if m >= 0:  # diagonal block: triangular mask
                        nc.vector.tensor_mul(pt[:, q0:], pt[:, q0:],
                                             mk_sb[:, m, q0:])
                    nc.tensor.matmul(
                        av[:, q0:],
                        lhsT=v_sbs[j // NQ][:, j % NQ,
                                            h * (DH + 1):(h + 1) * (DH + 1)],
                        rhs=pt[:, q0:],
                        start=(j == 0), stop=(j == nkb_c - 1),
                    )
                # normalize: A^T[f, q] = av[f, q] * (1 / denom[q]); broadcast
                # the reciprocal row over 64 partitions via a K=1 outer product
                rc = rcp.tile([1, SC], MM_DT, tag="rc", name="rc")
                with nc.allow_low_precision(
                        reason="f32r rounding of softmax recip is benign"):
                    nc.vector.reciprocal(rc[0:1, :], av[DH:DH + 1, :])
                rb_ps = pmx.tile([DH, SC], F32, tag="b", name="psb")
                nc.tensor.matmul(rb_ps[:], lhsT=ones_sb[0:1, :],
                                 rhs=rc[0:1, :], start=True, stop=True)
                rcb = rcp.tile([DH, SC], F32, tag="rcb", name="rcb")
                nc.vector.tensor_copy(rcb[:], rb_ps[:])
                nc.vector.tensor_mul(
                    at_sbs[c][et][er:er + DH, :], av[0:DH, :], rcb[:])

            def out_proj(c):
                for eb in range(D // P):
                    po = pmx.tile([P, SC], F32, tag="b", name="psb")
                    for ft in range(NEB):
                        nc.tensor.matmul(
                            po[:],
                            lhsT=wo_sbs[ft][:, eb * P:(eb + 1) * P],
                            rhs=at_sbs[c][ft][:],
                            start=(ft == 0), stop=(ft == NEB - 1),
                        )
                    ob = obp.tile([P, SC], F32, tag="ob", name="ob")
                    nc.vector.tensor_copy(ob[:], po[:])
                    # SWDGE queue: output stores run parallel to input HWDGE
                    dma = nc.gpsimd.dma_start if repeat == 1 else nc.sync.dma_start
                    dma(outT[eb * P:(eb + 1) * P, c * SC:(c + 1) * SC], ob[:])

            # ---- software-pipelined schedule over q-chunks: emit the
            # next chunk's projections before this chunk's attention so PE
            # has independent work while exp/normalize chains drain ----
            for c in range(NQ):
                project_qk(wk_sbs, bk_sb, kts, c)
                project_qk(wq_sbs, bq_sb, qts, c)
                project_v(c)
                if c > 0:
                    for h in range(HPC):
                        attention(h, c - 1)
                    out_proj(c - 1)
            for h in range(HPC):
                attention(h, NQ - 1)
            out_proj(NQ - 1)

    _split_multiwait(nc)
    return nc


_NC_CACHE = None


def kernel(**inputs):
    global _NC_CACHE
    if _NC_CACHE is None:
        _NC_CACHE = build_nc()
    nc = _NC_CACHE

    x = np.asarray(inputs["x"], np.float32)
    Wq = np.asarray(inputs["Wq"], np.float32)
    Wk = np.asarray(inputs["Wk"], np.float32)
    Wv = np.asarray(inputs["Wv"], np.float32)
    Wo = np.asarray(inputs["Wo"], np.float32)
    bq = np.asarray(inputs["bq"], np.float32)
    bk = np.asarray(inputs["bk"], np.float32)
    bv = np.asarray(inputs["bv"], np.float32)
    bo = np.asarray(inputs["bo"], np.float32)
    # The mask input is causal (tril ones) by construction; the kernel
    # hardcodes causal structure.

    xTs = [np.ascontiguousarray(x[b].T) for b in range(B)]
    in_maps = []
    for c in range(N_CORES):
        b, g = divmod(c, GROUPS)
        rows = slice(g * E, (g + 1) * E)
        in_maps.append({
            "xT": xTs[b],
            "wqT": np.ascontiguousarray(Wq[rows].T),
            "wkT": np.ascontiguousarray(Wk[rows].T),
            "wvT": np.ascontiguousarray(Wv[rows].T),
            "woT": np.ascontiguousarray(Wo[:, rows].T),
            "bq": np.ascontiguousarray(bq[rows]),
            "bk": np.ascontiguousarray(bk[rows]),
        })

    global _last_in_maps
    _last_in_maps = in_maps
    res = run_bass_kernel_spmd(nc, in_maps, list(range(N_CORES)))

    out = np.zeros((B, S, D), np.float32)
    for c in range(N_CORES):
        b = c // GROUPS
        out[b] += res.results[c]["outT"].T
    # bv enters only additively after softmax (rows of P sum to 1):
    # out += Wo @ bv; plus the output bias bo.
    out += (Wo @ bv + bo)[None, None, :]
    return out



# revision 8
# speedup vs baseline: 1.7284x; 1.7284x over previous
"""Multi-head causal self-attention (B=2, S=2048, D=1024, H=16) on 8 TRN2
NeuronCores via Bass/Tile.

Sharding: core c -> (batch b = c // 4, head-group g = c % 4). Each core
computes q/k/v projections for its 4 heads (256 of 1024 projection cols),
causal flash attention for those heads, and a partial output projection
(row-parallel over the head dim). Host sums the 4 partials per batch.

v3 vs the fp32r baseline (360us):
  * all matmul operands bf16 (fp32r measured 2 PE-cycles/moving-row on HW;
    bf16 is 1) -> PE busy time halves; input DMA bytes halve
  * K-projection bias dropped: a k-side bias shifts every score in a
    softmax row equally -> provably cancels (bv/bo were already host-side)
  * softmax 1/denom = exp(-ln(denom)) on ACT -- both functions live in one
    activation table set so there is no table-switch cost; kills the 3.4us
    iterative DVE reciprocal per head
  * exp fused over PAIRS of k-blocks ([128,1024] over a 2-bank PSUM score
    tile) -> halves ACT instruction overhead; score matmuls always write
    full 512 cols so the fused exp never sees stale PSUM garbage; causal
    masking via one bf16 tensor-tensor multiply per diagonal pair
  * flat software pipeline over (chunk, head) units: engines execute their
    streams in scheduler priority order (emission order), so attention is
    emitted first and next-chunk projection / prev-chunk output-projection
    slices are interleaved BETWEEN heads as PE filler; each unit's softmax
    normalization lags one unit so the ACT queue never stalls on the AV
    matmul chain
  * PE/ACT warmed up with dummy work during the initial DMA so the HAM
    clock-gate opens (4/8 -> 8/8) before the real matmuls

Device layouts (contraction dim on partitions everywhere):
  xT   [D, S]   : x[b].T, host-transposed
  Q^T/K^T [e, S]: head dim on partitions (2 heads per 128-partition tile)
  V    [k, e+1] : natural, with a ones column per head; the ones column turns
                  the AV^T matmul into (unnormalized AV^T, softmax denom) rows
  A^T  [e, S]   : produced directly by AV^T matmul, consumed as moving
                  operand of the output projection -> zero on-chip transposes
  outT [D, S]   : transposed partial output, host sums + transposes back
"""

from contextlib import ExitStack

import numpy as np
import ml_dtypes

import concourse.bass as bass
import concourse.mybir as mybir
import concourse.tile as tile
from concourse.bass_utils import run_bass_kernel_spmd

# Problem constants (hardcoded per harness contract).
B, S, D, NH, DH = 2, 2048, 1024, 16, 64
N_CORES = 8
GROUPS = 4                 # head-groups; cores per batch
HPC = NH // GROUPS         # heads per core = 4
E = HPC * DH               # per-core projection width = 256
P = 128                    # SBUF partitions
SC = 512                   # moving-operand chunk (q chunk)
ND = D // P                # 8 d-chunks
NEB = E // P               # 2 e-blocks per core
NQ = S // SC               # 4 q chunks
NKB = S // P               # 16 k blocks
SCALE = DH ** -0.5

F32 = mybir.dt.float32
BF16 = mybir.dt.bfloat16


def _split_multiwait(nc, max_waits=1):
    """This toolchain's walrus codegen accepts at most one sync-wait per
    instruction ("Too many sync wait commands"). Tile emits multi-wait
    instructions (notably the kernel-tail Drain). Keep the last wait (+ all
    updates) on the original instruction and hoist earlier waits onto
    single-wait Drains inserted before it on the same engine."""
    for f in nc.m.functions:
        for bb in f.blocks:
            new = []
            changed = False
            for inst in bb.instructions:
                si = inst.sync_info
                waits = list(si.on_wait) if si is not None and si.on_wait else []
                if len(waits) > max_waits:
                    for j, w in enumerate(waits[:-max_waits]):
                        d = mybir.InstDrain(name=f"{inst.name}-sw{j}", ins=[], outs=[])
                        d.engine = inst.engine
                        d.sync_info = mybir.SyncInfo(on_wait=[w], on_update=[])
                        new.append(d)
                    inst.sync_info = mybir.SyncInfo(
                        on_wait=waits[-max_waits:],
                        on_update=list(si.on_update) if si.on_update else [],
                    )
                    changed = True
                new.append(inst)
            if changed:
                bb.instructions = new


def build_nc():
    nc = bass.Bass("TRN2", target_bir_lowering=False, debug=False,
                   num_devices=N_CORES)

    xT = nc.dram_tensor("xT", [D, S], BF16, kind="ExternalInput")
    wqT = nc.dram_tensor("wqT", [D, E], BF16, kind="ExternalInput")
    wkT = nc.dram_tensor("wkT", [D, E], BF16, kind="ExternalInput")
    wvT = nc.dram_tensor("wvT", [D, E], BF16, kind="ExternalInput")
    woT = nc.dram_tensor("woT", [E, D], BF16, kind="ExternalInput")
    bq = nc.dram_tensor("bq", [E], F32, kind="ExternalInput")
    outT = nc.dram_tensor("outT", [D, S], F32, kind="ExternalOutput")

    AF = mybir.ActivationFunctionType
    with tile.TileContext(nc) as tc:
        with ExitStack() as ctx:
            const = ctx.enter_context(tc.tile_pool(name="const", bufs=1))

            # ---- persistent SBUF tensors (chunked for fine-grained deps) ----
            x_sbs = [const.tile([P, S], BF16, tag=f"x{i}", name=f"x{i}") for i in range(ND)]
            wq_sbs = [const.tile([P, E], BF16, tag=f"wq{i}", name=f"wq{i}") for i in range(ND)]
            wk_sbs = [const.tile([P, E], BF16, tag=f"wk{i}", name=f"wk{i}") for i in range(ND)]
            wv_sbs = [const.tile([P, E], BF16, tag=f"wv{i}", name=f"wv{i}") for i in range(ND)]
            wo_sbs = [const.tile([P, D], BF16, tag=f"wo{i}", name=f"wo{i}") for i in range(NEB)]
            bq_sb = const.tile([P, NEB], F32, tag="bq", name="bq")
            qts = [[const.tile([P, SC], BF16, tag=f"qt{e}{c}", name=f"qt{e}{c}") for c in range(NQ)]
                   for e in range(NEB)]
            kts = [[const.tile([P, SC], BF16, tag=f"kt{e}{c}", name=f"kt{e}{c}") for c in range(NQ)]
                   for e in range(NEB)]
            v_sbs = [const.tile([P, NQ, HPC * (DH + 1)], BF16, tag=f"v{i}", name=f"v{i}")
                     for i in range(NQ)]
            at_sbs = [[const.tile([P, SC], BF16, tag=f"at{i}{f}", name=f"at{i}{f}")
                       for f in range(NEB)] for i in range(NQ)]
            mk_sb = const.tile([P, NQ, SC], BF16, tag="mk", name="mk")
            ones_sb = const.tile([1, DH], BF16, tag="ones", name="ones")
            warm_sb = const.tile([P, SC], BF16, tag="warm", name="warm")

            # ---- input DMAs: HWDGE (sync) + SWDGE (gpsimd) queues, ordered
            # so chunk-0 projections can start after ~3us ----
            for di in range(ND):
                nc.sync.dma_start(wk_sbs[di][:], wkT[di * P:(di + 1) * P, :])
                nc.sync.dma_start(wq_sbs[di][:], wqT[di * P:(di + 1) * P, :])
            for cc in range(NQ):
                dma = nc.gpsimd.dma_start if cc % 2 else nc.sync.dma_start
                for di in range(ND):
                    dma(x_sbs[di][:, cc * SC:(cc + 1) * SC],
                        xT[di * P:(di + 1) * P, cc * SC:(cc + 1) * SC])
                if cc == 0:
                    for di in range(ND):
                        nc.gpsimd.dma_start(wv_sbs[di][:], wvT[di * P:(di + 1) * P, :])
            for ft in range(NEB):
                nc.sync.dma_start(wo_sbs[ft][:], woT[ft * P:(ft + 1) * P, :])
            nc.sync.dma_start(bq_sb[:], bq.rearrange("(n p) -> p n", p=P))

            # constants: ones + multiplicative causal masks
            tmp = ctx.enter_context(tc.tile_pool(name="tmp", bufs=1))
            one_f32 = tmp.tile([P, 1], F32, tag="onef", name="onef")
            nc.vector.memset(one_f32[:], 1.0)
            nc.vector.tensor_copy(ones_sb[:],
                                  one_f32[0:1, 0:1].broadcast_to([1, DH]))
            for cc in range(NQ):
                nc.vector.tensor_copy(
                    v_sbs[cc][:, :, DH::DH + 1],
                    one_f32[:, :, None].broadcast_to([P, NQ, HPC]))
            # mk[m][kk, qq] = 1.0 if kk + 128*m <= qq else 0.0
            mkf_sb = tmp.tile([P, NQ, SC], F32, tag="mkf", name="mkf")
            for m in range(NQ):
                nc.gpsimd.memset(mkf_sb[:, m, :], 1.0)
                nc.gpsimd.affine_select(
                    out=mkf_sb[:, m, :], in_=mkf_sb[:, m, :],
                    compare_op=mybir.AluOpType.is_ge, fill=0.0,
                    base=-(P * m), pattern=[[1, SC]], channel_multiplier=-1,
                )
            nc.vector.tensor_copy(mk_sb[:], mkf_sb[:])

            # PSUM pools (8 banks): psq = 2-bank score tiles x2, pav = AV
            # accumulators x2, pgen = shared 1-bank ring for projections /
            # out-proj / reciprocal-broadcast / warmup.
            psq = ctx.enter_context(tc.tile_pool(name="psq", bufs=2, space="PSUM"))
            pav = ctx.enter_context(tc.tile_pool(name="pav", bufs=2, space="PSUM"))
            pgen = ctx.enter_context(tc.tile_pool(name="pgen", bufs=2, space="PSUM"))
            ptp = ctx.enter_context(tc.tile_pool(name="ptp", bufs=4))
            rcp = ctx.enter_context(tc.tile_pool(name="rcp", bufs=2))
            obp = ctx.enter_context(tc.tile_pool(name="obp", bufs=3))

            # ---- engine warmup during the initial DMA wait ----
            warm_ps = pgen.tile([P, SC], F32, tag="g", name="gps")
            nc.vector.memset(warm_sb[:], 0.0)
            for i in range(10):
                nc.tensor.matmul(warm_ps[:], lhsT=warm_sb[:, :P], rhs=warm_sb[:],
                                 start=True, stop=True)
            warm_act = tmp.tile([P, 8], F32, tag="wact", name="warmact")
            nc.scalar.activation(warm_act[:], one_f32[:].broadcast_to([P, 8]),
                                 AF.Exp, scale=1.0)
            nc.scalar.activation(warm_act[:], one_f32[:].broadcast_to([P, 8]),
                                 AF.Ln)

            def project_qk(w_sbs, o_tiles, c, bias, eb):
                ps = pgen.tile([P, SC], F32, tag="g", name="gps")
                for di in range(ND):
                    nc.tensor.matmul(
                        ps[:],
                        lhsT=w_sbs[di][:, eb * P:(eb + 1) * P],
                        rhs=x_sbs[di][:, c * SC:(c + 1) * SC],
                        start=(di == 0), stop=(di == ND - 1),
                    )
                if bias:
                    nc.vector.tensor_scalar_add(
                        out=o_tiles[eb][c][:], in0=ps[:],
                        scalar1=bq_sb[:, eb:eb + 1])
                else:
                    nc.vector.tensor_copy(o_tiles[eb][c][:], ps[:])

            def project_v(c, kk):
                kb = c * NQ + kk
                ps = pgen.tile([P, SC], F32, tag="g", name="gps")
                for di in range(ND):
                    nc.tensor.matmul(
                        ps[:, :E],
                        lhsT=x_sbs[di][:, kb * P:(kb + 1) * P],
                        rhs=wv_sbs[di][:],
                        start=(di == 0), stop=(di == ND - 1),
                    )
                dst = v_sbs[c][:, kk, :].rearrange(
                    "p (h e) -> p h e", h=HPC)[:, :, :DH]
                nc.vector.tensor_copy(
                    dst, ps[:, :E].rearrange("p (h e) -> p h e", h=HPC))

            def proj_slice(c, h):
                if h == 0:
                    project_qk(wk_sbs, kts, c, False, 0)
                    project_qk(wk_sbs, kts, c, False, 1)
                elif h == 1:
                    project_qk(wq_sbs, qts, c, True, 0)
                    project_qk(wq_sbs, qts, c, True, 1)
                elif h == 2:
                    project_v(c, 0)
                    project_v(c, 1)
                else:
                    project_v(c, 2)
                    project_v(c, 3)

            avs = {}

            def attention(c, h):
                """Scores + exp + AV for one (chunk, head); normalization is
                emitted one unit later (normalize())."""
                et, er = h // 2, (h % 2) * DH
                av = pav.tile([DH + 1, SC], F32, tag="av", name="av")
                avs[(c, h)] = av
                nkb_c = NQ * (c + 1)
                npair = nkb_c // 2
                sq_tiles = {}
                pt_tiles = {}

                def finish_pair(g):
                    pt = pt_tiles.pop(g)
                    if 2 * g >= NQ * c:  # diagonal pair: triangular mask
                        m0 = 2 * g - NQ * c
                        nc.vector.tensor_mul(pt[:], pt[:],
                                             mk_sb[:, m0:m0 + 2, :])
                    for b2 in range(2):
                        j = 2 * g + b2
                        m = j - NQ * c
                        q0 = P * m if m > 0 else 0
                        nc.tensor.matmul(
                            av[:, q0:],
                            lhsT=v_sbs[j // NQ][:, j % NQ,
                                                h * (DH + 1):(h + 1) * (DH + 1)],
                            rhs=pt[:, b2, q0:],
                            start=(j == 0), stop=(j == nkb_c - 1),
                        )

                for g in range(npair):
                    # scores for k-blocks (2g, 2g+1): one 2-bank psum tile,
                    # always full 512 wide (no stale-psum garbage under the
                    # fused exp; diagonal future positions are real bounded
                    # scores, zeroed by the mask multiply after exp)
                    ps = psq.tile([P, 2, SC], F32, tag="sq", name="sq")
                    sq_tiles[g] = ps
                    for b2 in range(2):
                        j = 2 * g + b2
                        nc.tensor.matmul(
                            ps[:, b2, :],
                            lhsT=kts[et][j // NQ][er:er + DH,
                                                  (j % NQ) * P:(j % NQ + 1) * P],
                            rhs=qts[et][c][er:er + DH, :],
                            start=True, stop=True,
                        )
                    if g > 0:
                        finish_pair(g - 1)
                    pt = ptp.tile([P, 2, SC], BF16, tag="pt", name="pt")
                    pt_tiles[g] = pt
                    nc.scalar.activation(pt[:], sq_tiles.pop(g)[:], AF.Exp,
                                         scale=SCALE)
                finish_pair(npair - 1)

            def normalize(c, h):
                """1/denom = exp(-ln(denom)) on ACT, broadcast over the head
                dim via a PE outer product, apply on DVE."""
                et, er = h // 2, (h % 2) * DH
                av = avs.pop((c, h))
                ln = rcp.tile([1, SC], F32, tag="ln", name="ln")
                rc = rcp.tile([1, SC], BF16, tag="rc", name="rc")
                nc.scalar.activation(ln[0:1, :], av[DH:DH + 1, :], AF.Ln)
                with nc.allow_low_precision(
                        reason="bf16 softmax recip rounding is benign"):
                    nc.scalar.activation(rc[0:1, :], ln[0:1, :], AF.Exp,
                                         scale=-1.0)
                rb = pgen.tile([DH, SC], F32, tag="g", name="gps")
                nc.tensor.matmul(rb[:], lhsT=ones_sb[0:1, :], rhs=rc[0:1, :],
                                 start=True, stop=True)
                rcb = rcp.tile([DH, SC], F32, tag="rcb", name="rcb")
                nc.vector.tensor_copy(rcb[:], rb[:])
                nc.vector.tensor_mul(
                    at_sbs[c][et][er:er + DH, :], av[0:DH, :], rcb[:])

            def out_proj_group(c, ebs):
                for eb in ebs:
                    po = pgen.tile([P, SC], F32, tag="g", name="gps")
                    for ft in range(NEB):
                        nc.tensor.matmul(
                            po[:],
                            lhsT=wo_sbs[ft][:, eb * P:(eb + 1) * P],
                            rhs=at_sbs[c][ft][:],
                            start=(ft == 0), stop=(ft == NEB - 1),
                        )
                    ob = obp.tile([P, SC], F32, tag="ob", name="ob")
                    nc.vector.tensor_copy(ob[:], po[:])
                    nc.sync.dma_start(
                        outT[eb * P:(eb + 1) * P, c * SC:(c + 1) * SC], ob[:])

            # ---- flat pipeline over (chunk, head) units ----
            for h in range(HPC):
                proj_slice(0, h)
            units = [(c, h) for c in range(NQ) for h in range(HPC)]
            for i, (c, h) in enumerate(units):
                attention(c, h)
                if i > 0:
                    normalize(*units[i - 1])
                if c > 0:
                    out_proj_group(c - 1, [2 * h, 2 * h + 1])
                if c + 1 < NQ:
                    proj_slice(c + 1, h)
            normalize(*units[-1])
            out_proj_group(NQ - 1, list(range(D // P)))

    _split_multiwait(nc)
    return nc


_NC_CACHE = None
_last_in_maps = None


def kernel(**inputs):
    global _NC_CACHE, _last_in_maps
    if _NC_CACHE is None:
        _NC_CACHE = build_nc()
    nc = _NC_CACHE

    bf = ml_dtypes.bfloat16
    x = np.asarray(inputs["x"], np.float32)
    Wq = np.asarray(inputs["Wq"], np.float32)
    Wk = np.asarray(inputs["Wk"], np.float32)
    Wv = np.asarray(inputs["Wv"], np.float32)
    Wo = np.asarray(inputs["Wo"], np.float32)
    bq = np.asarray(inputs["bq"], np.float32)
    bv = np.asarray(inputs["bv"], np.float32)
    bo = np.asarray(inputs["bo"], np.float32)
    # The mask input is causal (tril ones) by construction; the kernel
    # hardcodes causal structure. bk is dropped: a k-side bias adds the
    # same offset to every score within a softmax row, so it cancels.

    xTs = [np.ascontiguousarray(x[b].T.astype(bf)) for b in range(B)]
    in_maps = []
    for c in range(N_CORES):
        b, g = divmod(c, GROUPS)
        rows = slice(g * E, (g + 1) * E)
        in_maps.append({
            "xT": xTs[b],
            "wqT": np.ascontiguousarray(Wq[rows].T.astype(bf)),
            "wkT": np.ascontiguousarray(Wk[rows].T.astype(bf)),
            "wvT": np.ascontiguousarray(Wv[rows].T.astype(bf)),
            "woT": np.ascontiguousarray(Wo[:, rows].T.astype(bf)),
            "bq": np.ascontiguousarray(bq[rows]),
        })

    _last_in_maps = in_maps
    res = run_bass_kernel_spmd(nc, in_maps, list(range(N_CORES)))

    out = np.zeros((B, S, D), np.float32)
    for c in range(N_CORES):
        b = c // GROUPS
        out[b] += res.results[c]["outT"].T
    # bv enters only additively after softmax (rows of P sum to 1):
    # out += Wo @ bv; plus the output bias bo.
    out += (Wo @ bv + bo)[None, None, :]
    return out


# revision 9
# speedup vs baseline: 1.7554x; 1.0156x over previous
"""Multi-head causal self-attention (B=2, S=2048, D=1024, H=16) on 8 TRN2
NeuronCores via Bass/Tile.

Sharding: core c -> (batch b = c // 4, head-group g = c % 4). Each core
computes q/k/v projections for its 4 heads (256 of 1024 projection cols),
causal flash attention for those heads, and a partial output projection
(row-parallel over the head dim). Host sums the 4 partials per batch.

v4 (baseline 360us -> v3 208us -> this):
  * all matmul operands bf16 (fp32r measured 2 PE-cycles/moving-row on HW;
    bf16 is 1); K-projection bias dropped (cancels in softmax); bv/bo on host
  * HEAD-PAIR PACKED scores: the two heads of an e-block live on partition
    halves 0:64 / 64:128, so their K=64 score matmuls target different PE
    row-groups and run concurrently (tile_position auto-derived from the
    lhsT base partition) -> score time halves
  * one [128, 2, 512] score PSUM tile per k-block (slot per head); exp fused
    across both heads in one ACT instruction; diagonal blocks trimmed to the
    causally-live columns in scores/exp/mask/AV
  * softmax 1/denom = exp(-ln(denom)) on ACT (one table set, no switches),
    batched over the head pair via a shared [65, 2, 512] AV accumulator
  * flat software pipeline over (chunk, e-block) units: normalization of the
    previous unit is emitted first (frees the AV tile early), then attention,
    then prev-chunk out-projection and next-chunk projection slices as PE
    filler; engines drain their streams in priority(=emission) order
  * input DMA ordered so K-proj of chunk 0 starts after ~2.5us (wk then the
    first 512 x-columns split across both DMA queues); output DMA alternates
    between the sync HWDGE and gpsimd SWDGE queues
  * PE/ACT warmed up with dummy work during the initial DMA so the HAM
    clock-gate opens (4/8 -> 8/8) before the real matmuls

Device layouts (contraction dim on partitions everywhere):
  xT   [D, S]   : x[b].T, host-transposed
  Q^T/K^T [e, S]: head dim on partitions (2 heads per 128-partition tile)
  V    [k, e+1] : natural, with a ones column per head; the ones column turns
                  the AV^T matmul into (unnormalized AV^T, softmax denom) rows
  A^T  [e, S]   : produced directly by AV^T matmul, consumed as moving
                  operand of the output projection -> zero on-chip transposes
  outT [D, S]   : transposed partial output, host sums + transposes back
"""

from contextlib import ExitStack

import numpy as np
import ml_dtypes

import concourse.bass as bass
import concourse.mybir as mybir
import concourse.tile as tile
from concourse.bass_utils import run_bass_kernel_spmd

# Problem constants (hardcoded per harness contract).
B, S, D, NH, DH = 2, 2048, 1024, 16, 64
N_CORES = 8
GROUPS = 4                 # head-groups; cores per batch
HPC = NH // GROUPS         # heads per core = 4
E = HPC * DH               # per-core projection width = 256
P = 128                    # SBUF partitions
SC = 512                   # moving-operand chunk (q chunk)
ND = D // P                # 8 d-chunks
NEB = E // P               # 2 e-blocks per core
NQ = S // SC               # 4 q chunks
NKB = S // P               # 16 k blocks
SCALE = DH ** -0.5

F32 = mybir.dt.float32
BF16 = mybir.dt.bfloat16


def _split_multiwait(nc, max_waits=1):
    """This toolchain's walrus codegen accepts at most one sync-wait per
    instruction ("Too many sync wait commands"). Tile emits multi-wait
    instructions (notably the kernel-tail Drain). Keep the last wait (+ all
    updates) on the original instruction and hoist earlier waits onto
    single-wait Drains inserted before it on the same engine."""
    for f in nc.m.functions:
        for bb in f.blocks:
            new = []
            changed = False
            for inst in bb.instructions:
                si = inst.sync_info
                waits = list(si.on_wait) if si is not None and si.on_wait else []
                if len(waits) > max_waits:
                    for j, w in enumerate(waits[:-max_waits]):
                        d = mybir.InstDrain(name=f"{inst.name}-sw{j}", ins=[], outs=[])
                        d.engine = inst.engine
                        d.sync_info = mybir.SyncInfo(on_wait=[w], on_update=[])
                        new.append(d)
                    inst.sync_info = mybir.SyncInfo(
                        on_wait=waits[-max_waits:],
                        on_update=list(si.on_update) if si.on_update else [],
                    )
                    changed = True
                new.append(inst)
            if changed:
                bb.instructions = new


def build_nc():
    nc = bass.Bass("TRN2", target_bir_lowering=False, debug=False,
                   num_devices=N_CORES)

    xT = nc.dram_tensor("xT", [D, S], BF16, kind="ExternalInput")
    wqT = nc.dram_tensor("wqT", [D, E], BF16, kind="ExternalInput")
    wkT = nc.dram_tensor("wkT", [D, E], BF16, kind="ExternalInput")
    wvT = nc.dram_tensor("wvT", [D, E], BF16, kind="ExternalInput")
    woT = nc.dram_tensor("woT", [E, D], BF16, kind="ExternalInput")
    bq = nc.dram_tensor("bq", [E], F32, kind="ExternalInput")
    outT = nc.dram_tensor("outT", [D, S], F32, kind="ExternalOutput")

    AF = mybir.ActivationFunctionType
    with tile.TileContext(nc) as tc:
        with ExitStack() as ctx:
            const = ctx.enter_context(tc.tile_pool(name="const", bufs=1))

            # ---- persistent SBUF tensors (chunked for fine-grained deps) ----
            x_sbs = [const.tile([P, S], BF16, tag=f"x{i}", name=f"x{i}") for i in range(ND)]
            wq_sbs = [const.tile([P, E], BF16, tag=f"wq{i}", name=f"wq{i}") for i in range(ND)]
            wk_sbs = [const.tile([P, E], BF16, tag=f"wk{i}", name=f"wk{i}") for i in range(ND)]
            wv_sbs = [const.tile([P, E], BF16, tag=f"wv{i}", name=f"wv{i}") for i in range(ND)]
            wo_sbs = [const.tile([P, D], BF16, tag=f"wo{i}", name=f"wo{i}") for i in range(NEB)]
            bq_sb = const.tile([P, NEB], F32, tag="bq", name="bq")
            qts = [[const.tile([P, SC], BF16, tag=f"qt{e}{c}", name=f"qt{e}{c}") for c in range(NQ)]
                   for e in range(NEB)]
            kts = [[const.tile([P, SC], BF16, tag=f"kt{e}{c}", name=f"kt{e}{c}") for c in range(NQ)]
                   for e in range(NEB)]
            v_sbs = [const.tile([P, NQ, HPC * (DH + 1)], BF16, tag=f"v{i}", name=f"v{i}")
                     for i in range(NQ)]
            at_sbs = [[const.tile([P, SC], BF16, tag=f"at{i}{f}", name=f"at{i}{f}")
                       for f in range(NEB)] for i in range(NQ)]
            # mk2[m][b2][kk, qq]: causal mask for diagonal-offset m, replicated
            # in the middle dim so one TT covers both heads of a pair
            mk2_sb = const.tile([P, NQ, 2, SC], BF16, tag="mk2", name="mk2")
            ones_sb = const.tile([1, DH], BF16, tag="ones", name="ones")
            warm_sb = const.tile([P, SC], BF16, tag="warm", name="warm")

            # ---- input DMAs: ordered so chunk-0 K-projection can start
            # ~2.5us in. sync HWDGE: wk, x[:, :512] even d-chunks, wq, then
            # the rest; gpsimd SWDGE: x[:, :512] odd d-chunks, wv, x rest ----
            for di in range(ND):
                nc.sync.dma_start(wk_sbs[di][:], wkT[di * P:(di + 1) * P, :])
            for di in range(ND):
                dma = nc.sync.dma_start if di % 2 == 0 else nc.gpsimd.dma_start
                dma(x_sbs[di][:, 0:SC], xT[di * P:(di + 1) * P, 0:SC])
            for di in range(ND):
                nc.sync.dma_start(wq_sbs[di][:], wqT[di * P:(di + 1) * P, :])
                nc.gpsimd.dma_start(wv_sbs[di][:], wvT[di * P:(di + 1) * P, :])
            for cc in range(1, NQ):
                dma = nc.gpsimd.dma_start if cc % 2 else nc.sync.dma_start
                for di in range(ND):
                    dma(x_sbs[di][:, cc * SC:(cc + 1) * SC],
                        xT[di * P:(di + 1) * P, cc * SC:(cc + 1) * SC])
            for ft in range(NEB):
                nc.sync.dma_start(wo_sbs[ft][:], woT[ft * P:(ft + 1) * P, :])
            nc.sync.dma_start(bq_sb[:], bq.rearrange("(n p) -> p n", p=P))

            # constants: ones + v ones-columns + causal masks
            tmp = ctx.enter_context(tc.tile_pool(name="tmp", bufs=1))
            one_f32 = tmp.tile([P, 1], F32, tag="onef", name="onef")
            nc.vector.memset(one_f32[:], 1.0)
            nc.vector.tensor_copy(ones_sb[:],
                                  one_f32[0:1, 0:1].broadcast_to([1, DH]))
            for cc in range(NQ):
                nc.vector.tensor_copy(
                    v_sbs[cc][:, :, DH::DH + 1],
                    one_f32[:, :, None].broadcast_to([P, NQ, HPC]))
            mkf_sb = tmp.tile([P, NQ, SC], F32, tag="mkf", name="mkf")
            for m in range(NQ):
                nc.gpsimd.memset(mkf_sb[:, m, :], 1.0)
                nc.gpsimd.affine_select(
                    out=mkf_sb[:, m, :], in_=mkf_sb[:, m, :],
                    compare_op=mybir.AluOpType.is_ge, fill=0.0,
                    base=-(P * m), pattern=[[1, SC]], channel_multiplier=-1,
                )
            for b2 in range(2):
                nc.vector.tensor_copy(mk2_sb[:, :, b2, :], mkf_sb[:])

            # PSUM pools (8 banks): psq = 2-bank score tiles x2, pav = one
            # 2-bank AV accumulator (slot per head), pgen = shared 1-bank
            # ring for projections / out-proj / recip-broadcast / warmup.
            psq = ctx.enter_context(tc.tile_pool(name="psq", bufs=2, space="PSUM"))
            pav = ctx.enter_context(tc.tile_pool(name="pav", bufs=1, space="PSUM"))
            pgen = ctx.enter_context(tc.tile_pool(name="pgen", bufs=2, space="PSUM"))
            ptp = ctx.enter_context(tc.tile_pool(name="ptp", bufs=4))
            rcp = ctx.enter_context(tc.tile_pool(name="rcp", bufs=2))
            obp = ctx.enter_context(tc.tile_pool(name="obp", bufs=3))

            # ---- engine warmup during the initial DMA wait ----
            warm_ps = pgen.tile([P, SC], F32, tag="g", name="gps")
            nc.vector.memset(warm_sb[:], 0.0)
            for i in range(14):
                nc.tensor.matmul(warm_ps[:], lhsT=warm_sb[:, :P], rhs=warm_sb[:],
                                 start=True, stop=True)
            warm_act = tmp.tile([P, 8], F32, tag="wact", name="warmact")
            nc.scalar.activation(warm_act[:], one_f32[:].broadcast_to([P, 8]),
                                 AF.Exp, scale=1.0)
            nc.scalar.activation(warm_act[:], one_f32[:].broadcast_to([P, 8]),
                                 AF.Ln)

            def project_qk(w_sbs, o_tiles, c, bias, eb):
                ps = pgen.tile([P, SC], F32, tag="g", name="gps")
                for di in range(ND):
                    nc.tensor.matmul(
                        ps[:],
                        lhsT=w_sbs[di][:, eb * P:(eb + 1) * P],
                        rhs=x_sbs[di][:, c * SC:(c + 1) * SC],
                        start=(di == 0), stop=(di == ND - 1),
                    )
                if bias:
                    nc.vector.tensor_scalar_add(
                        out=o_tiles[eb][c][:], in0=ps[:],
                        scalar1=bq_sb[:, eb:eb + 1])
                else:
                    nc.vector.tensor_copy(o_tiles[eb][c][:], ps[:])

            def project_v(c, kk):
                kb = c * NQ + kk
                ps = pgen.tile([P, SC], F32, tag="g", name="gps")
                for di in range(ND):
                    nc.tensor.matmul(
                        ps[:, :E],
                        lhsT=x_sbs[di][:, kb * P:(kb + 1) * P],
                        rhs=wv_sbs[di][:],
                        start=(di == 0), stop=(di == ND - 1),
                    )
                dst = v_sbs[c][:, kk, :].rearrange(
                    "p (h e) -> p h e", h=HPC)[:, :, :DH]
                nc.vector.tensor_copy(
                    dst, ps[:, :E].rearrange("p (h e) -> p h e", h=HPC))

            def proj_slice(c, et):
                if et == 0:
                    project_qk(wk_sbs, kts, c, False, 0)
                    project_qk(wk_sbs, kts, c, False, 1)
                    project_qk(wq_sbs, qts, c, True, 0)
                    project_qk(wq_sbs, qts, c, True, 1)
                else:
                    for kk in range(NQ):
                        project_v(c, kk)

            avs = {}

            def attention(c, et):
                """Scores + exp + AV for a (chunk, head-pair) unit. The two
                heads sit on partition halves of the kt/qt tiles, so their
                K=64 score matmuls run on different PE row-groups
                concurrently. One [P, 2, SC] score tile per k-block; exp and
                the diagonal mask cover both heads in single instructions."""
                av = pav.tile([DH + 1, 2, SC], F32, tag="av", name="av")
                avs[(c, et)] = av
                nkb_c = NQ * (c + 1)
                pt_tiles = {}

                def finish(j):
                    pt = pt_tiles.pop(j)
                    m = j - NQ * c
                    q0 = P * m if m > 0 else 0
                    if m >= 0:  # diagonal block: triangular mask, both heads
                        nc.vector.tensor_mul(pt[:, :, q0:], pt[:, :, q0:],
                                             mk2_sb[:, m, :, q0:])
                    for b2 in range(2):
                        h = 2 * et + b2
                        nc.tensor.matmul(
                            av[:, b2, q0:],
                            lhsT=v_sbs[j // NQ][:, j % NQ,
                                                h * (DH + 1):(h + 1) * (DH + 1)],
                            rhs=pt[:, b2, q0:],
                            start=(j == 0), stop=(j == nkb_c - 1),
                        )

                for j in range(nkb_c):
                    m = j - NQ * c
                    q0 = P * m if m > 0 else 0
                    ps = psq.tile([P, 2, SC], F32, tag="sq", name="sq")
                    for b2 in range(2):
                        er = b2 * DH
                        nc.tensor.matmul(
                            ps[:, b2, q0:],
                            lhsT=kts[et][j // NQ][er:er + DH,
                                                  (j % NQ) * P:(j % NQ + 1) * P],
                            rhs=qts[et][c][er:er + DH, q0:],
                            start=True, stop=True,
                        )
                    if j > 0:
                        finish(j - 1)
                    pt = ptp.tile([P, 2, SC], BF16, tag="pt", name="pt")
                    pt_tiles[j] = pt
                    nc.scalar.activation(pt[:, :, q0:], ps[:, :, q0:], AF.Exp,
                                         scale=SCALE)
                finish(nkb_c - 1)

            def normalize(c, et):
                """1/denom = exp(-ln(denom)) on ACT, batched over the head
                pair; broadcast over the head dim via PE outer products."""
                av = avs.pop((c, et))
                ln2 = rcp.tile([1, 2, SC], F32, tag="ln", name="ln")
                rc2 = rcp.tile([1, 2, SC], BF16, tag="rc", name="rc")
                nc.scalar.activation(ln2[0:1, :, :], av[DH:DH + 1, :, :], AF.Ln)
                with nc.allow_low_precision(
                        reason="bf16 softmax recip rounding is benign"):
                    nc.scalar.activation(rc2[0:1, :, :], ln2[0:1, :, :], AF.Exp,
                                         scale=-1.0)
                for b2 in range(2):
                    rb = pgen.tile([DH, SC], F32, tag="g", name="gps")
                    nc.tensor.matmul(rb[:], lhsT=ones_sb[0:1, :],
                                     rhs=rc2[0:1, b2, :], start=True, stop=True)
                    rcb = rcp.tile([DH, SC], F32, tag="rcb", name="rcb")
                    nc.vector.tensor_copy(rcb[:], rb[:])
                    er = b2 * DH
                    nc.vector.tensor_mul(
                        at_sbs[c][et][er:er + DH, :], av[0:DH, b2, :], rcb[:])

            def out_proj_group(c, ebs):
                for eb in ebs:
                    po = pgen.tile([P, SC], F32, tag="g", name="gps")
                    for ft in range(NEB):
                        nc.tensor.matmul(
                            po[:],
                            lhsT=wo_sbs[ft][:, eb * P:(eb + 1) * P],
                            rhs=at_sbs[c][ft][:],
                            start=(ft == 0), stop=(ft == NEB - 1),
                        )
                    ob = obp.tile([P, SC], F32, tag="ob", name="ob")
                    nc.vector.tensor_copy(ob[:], po[:])
                    dma = nc.sync.dma_start if eb % 2 == 0 else nc.gpsimd.dma_start
                    dma(outT[eb * P:(eb + 1) * P, c * SC:(c + 1) * SC], ob[:])

            # ---- flat pipeline over (chunk, e-block) units ----
            for et in range(NEB):
                proj_slice(0, et)
            units = [(c, et) for c in range(NQ) for et in range(NEB)]
            for i, (c, et) in enumerate(units):
                if i > 0:
                    normalize(*units[i - 1])
                attention(c, et)
                if c > 0:
                    out_proj_group(c - 1, [4 * et + k for k in range(4)])
                if c + 1 < NQ:
                    proj_slice(c + 1, et)
            normalize(*units[-1])
            out_proj_group(NQ - 1, list(range(D // P)))

    _split_multiwait(nc)
    return nc


_NC_CACHE = None
_last_in_maps = None


def kernel(**inputs):
    global _NC_CACHE, _last_in_maps
    if _NC_CACHE is None:
        _NC_CACHE = build_nc()
    nc = _NC_CACHE

    bf = ml_dtypes.bfloat16
    x = np.asarray(inputs["x"], np.float32)
    Wq = np.asarray(inputs["Wq"], np.float32)
    Wk = np.asarray(inputs["Wk"], np.float32)
    Wv = np.asarray(inputs["Wv"], np.float32)
    Wo = np.asarray(inputs["Wo"], np.float32)
    bq = np.asarray(inputs["bq"], np.float32)
    bv = np.asarray(inputs["bv"], np.float32)
    bo = np.asarray(inputs["bo"], np.float32)
    # The mask input is causal (tril ones) by construction; the kernel
    # hardcodes causal structure. bk is dropped: a k-side bias adds the
    # same offset to every score within a softmax row, so it cancels.

    xTs = [np.ascontiguousarray(x[b].T.astype(bf)) for b in range(B)]
    in_maps = []
    for c in range(N_CORES):
        b, g = divmod(c, GROUPS)
        rows = slice(g * E, (g + 1) * E)
        in_maps.append({
            "xT": xTs[b],
            "wqT": np.ascontiguousarray(Wq[rows].T.astype(bf)),
            "wkT": np.ascontiguousarray(Wk[rows].T.astype(bf)),
            "wvT": np.ascontiguousarray(Wv[rows].T.astype(bf)),
            "woT": np.ascontiguousarray(Wo[:, rows].T.astype(bf)),
            "bq": np.ascontiguousarray(bq[rows]),
        })

    _last_in_maps = in_maps
    res = run_bass_kernel_spmd(nc, in_maps, list(range(N_CORES)))

    out = np.zeros((B, S, D), np.float32)
    for c in range(N_CORES):
        b = c // GROUPS
        out[b] += res.results[c]["outT"].T
    # bv enters only additively after softmax (rows of P sum to 1):
    # out += Wo @ bv; plus the output bias bo.
    out += (Wo @ bv + bo)[None, None, :]
    return out
